# revision 12
# baseline (speedup 1.0000x reference)
"""AxonLIFNode forward on 8 Trainium2 NeuronCores.

Reference recurrence (per element, sequential over T):
    mem   = mem + (x_t + V_RESET - mem) / TAU        # V_RESET=0, TAU=2
    spike = (mem - V_TH > 0)                         # V_TH=1, {0.0, 1.0}
    mem   = (1 - spike) * mem + V_RESET * spike      # reset to 0 on spike
    out_i = out_i * sigmoid(w) + spike               # axon current (w=0 -> 0.5)
    outputs: (spike, out_i), both [B, T, N] f32

Strategy: data-parallel over the batch axis (B=64 -> 8 per core). Per core the
32768 independent series are laid out as 128 partitions x 256 free elements.
Both recurrences live in linear SBUF buffers with one slot per timestep
(slot 0 = zero initial state) so a single DVE instruction advances a GROUP of
timesteps: for the group's later timesteps the input stream reads the values
the same instruction wrote exactly F=256 elements earlier in the stream --
far beyond the DVE's 8-slice pipeline depth, so the within-instruction RAW is
safe (this is the same pattern the per-group OI op used; here it is applied
to BOTH chains):

    m1_buf[:, t+1, :] = prev + (x_t - prev) * 0.5,  prev = m1_t * (m1_t <= 1)
    oi_buf[:, t+1, :] = oi_t * inv_tau + (m1_{t+1} > 1)

which is bit-exact vs. the reference ordering for m1/spikes (each ALU stage is
one IEEE f32 rounding; *0.5 == /2 exactly). Group sizes taper [1,1,2,4,...]
at the start (compute starts after the first 128 KiB of input) and
[...,4,2,2] at the end (short final dependency chain before the last store).

HBM traffic per core is the wall: X in (8 MiB f32, must stay f32 -- any input
rounding flips spikes), spikes out as fp8-e4m3 ({0,1} lossless, 2 MiB), and
the axon current out as fp16 (4 MiB; the recurrence feedback also runs
through the fp16 buffer, worst-case relative error ~1e-3 << tolerance).
Spikes are produced off the critical path on the Scalar(ACT) engine with a
saturated sigmoid (exact {0,1}, see _build). X streams in on the SP HWDGE
ring; spikes + early oi groups stream out on the ACT ring while late oi
groups move to the SP ring once inputs finish.
"""

import numpy as np

import concourse.bacc as bacc
import concourse.mybir as mybir
import concourse.dve_ops as dve_ops
from concourse.dve_ops import DveOp
from concourse.dve_spec import Spec, Src0, Src1, C0, C1, lower
from concourse.dve_uop import DveOpSpec
from concourse.tile import TileContext
from concourse.bass_utils import run_bass_kernel_spmd

# Problem shape (hardcoded per harness contract).
B, T, N = 64, 64, 4096
CORES = 8
BS = B // CORES          # batches per core
P = 128                  # SBUF partitions
J = 16                   # n-chunks per batch: BS * J == P
F = N // J               # free elements per partition per timestep (256)
SPK_SCALE = 1.0e9        # sigmoid saturation trick scale (see _build)

# Input DMA batches: EXACTLY 8 so each takes one of the 8 global HWDGE
# completion-semaphore lanes at tick 1 -- a 9th+ HWDGE DMA gets a dispatch
# wait for its lane's previous user, and those waits resolve sluggishly
# (measured ~2-4us beyond data completion), serializing the input stream.
BATCH_STARTS = [0, 4, 8, 16, 24, 32, 40, 48, 56, 64]
# M1 group sizes (one DVE instruction each; must nest within input batches).
# Small ramp so compute starts early; small tail so the last chain is short.
GROUPS = [4, 4, 8, 8, 8, 8, 8, 8, 4, 2, 2]
assert sum(GROUPS) == T
# OI / spike / store spans [a, b): emitted once m_buf slots a+1..b are ready.
SPANS = [(0, 4), (4, 8), (8, 24), (24, 40), (40, 48), (48, 56), (56, 62),
         (62, 64)]


def _register_op(name: str, spec: Spec) -> DveOp:
    """Register a custom DVE op in the global registry with a computed sha."""
    for op in dve_ops.OPS:
        if op.name == name:
            return op
    row = dve_ops._CUSTOM_DVE_ROW_BASE + len(dve_ops.OPS)
    assert row < 0x20, "custom-DVE opcode rows exhausted"
    shas = {}
    for ver in ("v3", "v4"):
        uops = lower(spec, ver=ver)
        shas[ver] = DveOpSpec(name=name, opcode=row, uops=uops, rd1_en=True).sha(ver)
    op = DveOp(name, spec, subdim=False, uops_sha=shas)
    dve_ops._SUB_OPCODE_FOR_NAME[name] = row
    dve_ops.OPS.append(op)
    dve_ops.CUSTOM_DVE_SPECS[name] = spec
    return op


def _lif_ops() -> tuple[DveOp, DveOp]:
    """LIF_M1: m1_t from (x_t, m1_{t-1}); LIF_OI: oi_t from (oi_{t-1}, m1_t).

    LIF_M1: out = prev + (Src0 - prev) * C0, prev = Src1 * (Src1 <= C1)
    LIF_OI: out = Src0 * C0 + (Src1 > C1)
    Each ALU stage is one IEEE f32 rounding; bit-exact vs the reference.
    """
    keep = Src1 <= C1
    prev = Src1 * keep
    m1 = _register_op(
        "LIF_M1_ANT",
        Spec(
            body=prev + (Src0 - prev) * C0,
            reference=lambda in0, in1, s0, s1, imm2: (
                (p := (in1 * (in1 <= s1)).astype(np.float32))
                + (in0 - p) * np.float32(s0)
            ).astype(np.float32),
        ),
    )
    oi = _register_op(
        "LIF_OI_ANT",
        Spec(
            body=Src0 * C0 + (Src1 > C1),
            reference=lambda in0, in1, s0, s1, imm2: (
                in0 * np.float32(s0) + (in1 > s1)
            ).astype(np.float32),
        ),
    )
    return m1, oi


_nc_cache: dict = {}


def _build(inv_tau: float):
    """Trace + compile the per-core Bass program (SPMD: same NEFF, 8 cores)."""
    key = float(inv_tau)
    if key in _nc_cache:
        return _nc_cache[key]

    lif_m1, lif_oi = _lif_ops()
    f32 = mybir.dt.float32
    f16 = mybir.dt.float16
    fp8 = mybir.dt.float8e4

    nc = bacc.Bacc(
        "TRN2",
        target_bir_lowering=False,
        debug=False,
        enable_asserts=False,
        num_devices=CORES,
    )
    # Host pre-transposes each core's shard to [(b j) = 128, T, F] contiguous,
    # so every DMA is a 3-dim AP with a contiguous run per partition.
    x_r = nc.dram_tensor("x", [P, T, F], f32, kind="ExternalInput").ap()
    # Spikes are exactly {0.0, 1.0}: store as fp8-e4m3 (lossless) to cut the
    # HBM write traffic 4x; the host upcasts to f32.
    spk_r = nc.dram_tensor("spk", [P, T, F], fp8, kind="ExternalOutput").ap()
    # Axon current as fp16: rel err ~1e-3 (feedback-stable: error recurrence
    # delta_t = 0.5*delta_{t-1} + rounding), host upcasts to f32.
    oi_r = nc.dram_tensor("oi", [P, T, F], f16, kind="ExternalOutput").ap()

    starts = np.cumsum([0] + GROUPS).tolist()

    with TileContext(nc) as tc:
        with (
            tc.tile_pool(name="const", bufs=1) as cpool,
            tc.tile_pool(name="xin", bufs=1) as xpool,
            tc.tile_pool(name="sout", bufs=6) as spool,
            tc.tile_pool(name="state", bufs=1) as mpool,
        ):
            # Spike via one ACT op: sigmoid(S*m1 - (S + 64)) with S = 1e9.
            # fl(S*m1) quantizes to a 64-ulp grid around S, so the argument is
            # always <= -64 (no spike, incl. m1 == V_TH exactly -> -64) or
            # >= +64 (spike); sigmoid saturates to 0.0 / 1.0 there.
            spk_bias = cpool.tile([P, 1], f32)
            nc.gpsimd.memset(spk_bias[:], -(SPK_SCALE + 64.0))

            # Linear state buffers: slot 0 is the zero initial state, value
            # for timestep t lives at slot t+1.
            m_buf = mpool.tile([P, T + 1, F], f32)
            oi_buf = mpool.tile([P, T + 1, F], f16)
            nc.gpsimd.memset(m_buf[:, 0, :], 0.0)
            nc.gpsimd.memset(oi_buf[:, 0, :], 0.0)

            # Per-batch X tiles. All 8 input DMAs are emitted at the highest
            # scheduler priority so they claim the 8 HWDGE lanes at tick 1
            # (zero dispatch waits) and stream FIFO on the SP ring.
            x_tiles = [
                xpool.tile(
                    [P, BATCH_STARTS[i + 1] - BATCH_STARTS[i], F],
                    f32,
                    name=f"x_{BATCH_STARTS[i]}",
                    bufs=1,
                )
                for i in range(len(BATCH_STARTS) - 1)
            ]
            # SDMA engines round-robin across ALL queued DMAs at packet
            # granularity, so enqueueing every batch at once makes the batch
            # the DVE is waiting on share the wire with batches needed much
            # later. Pace the dispatch of later batches with manual
            # timestamps so early batches drain at full rate.
            # Ramp batches alternate between the two HWDGE rings so they
            # drain concurrently (the descriptor-size-limited rate of one
            # small transfer is well under the wire rate).
            x_dispatch_ms = {4: 0.016, 5: 0.018, 6: 0.020, 7: 0.022,
                             8: 0.024}
            with tc.high_priority():
                for bi in range(len(x_tiles)):
                    eng = nc.scalar if bi in (1, 3) else nc.sync
                    with tc.tile_wait_until(
                        x_dispatch_ms.get(bi, 0), enable=bi in x_dispatch_ms
                    ):
                        eng.dma_start(
                            out=x_tiles[bi][:],
                            in_=x_r[:, BATCH_STARTS[bi] : BATCH_STARTS[bi + 1], :],
                        )

            def x_slice(t0, g):
                for bi in range(len(x_tiles)):
                    if BATCH_STARTS[bi] <= t0 and t0 + g <= BATCH_STARTS[bi + 1]:
                        lo = t0 - BATCH_STARTS[bi]
                        return x_tiles[bi][:, lo : lo + g, :]
                raise AssertionError((t0, g))

            span_idx = 0
            for gi, g in enumerate(GROUPS):
                t0 = starts[gi]
                # DVE: advance the membrane chain g timesteps in one op.
                nc.vector._custom_dve(
                    lif_m1,
                    out=m_buf[:, t0 + 1 : t0 + 1 + g, :],
                    in0=x_slice(t0, g),
                    in1=m_buf[:, t0 : t0 + g, :],
                    s0=0.5,      # 1/TAU
                    s1=1.0,      # V_TH
                )
                while span_idx < len(SPANS) and SPANS[span_idx][1] == t0 + g:
                    a, b = SPANS[span_idx]
                    span_idx += 1
                    # ACT: spike = sigmoid(S*m1 - (S+64)) in {0, 1}, one
                    # op/span, written as fp8 (exact for {0,1}).
                    s_t = spool.tile([P, b - a, F], fp8)
                    nc.scalar.activation(
                        out=s_t[:],
                        in_=m_buf[:, a + 1 : b + 1, :],
                        func=mybir.ActivationFunctionType.Sigmoid,
                        bias=spk_bias[:],
                        scale=SPK_SCALE,
                    )
                    nc.scalar.dma_start(out=spk_r[:, a:b, :], in_=s_t[:])
                    # DVE: advance the axon-current chain b-a timesteps in
                    # one op (fp16 in/out; ALU math is fp32 internally).
                    nc.vector._custom_dve(
                        lif_oi,
                        out=oi_buf[:, a + 1 : b + 1, :],
                        in0=oi_buf[:, a : b, :],
                        in1=m_buf[:, a + 1 : b + 1, :],
                        s0=inv_tau,
                        s1=1.0,
                    )
                    oi_eng = nc.scalar if b <= 40 else nc.sync
                    oi_eng.dma_start(
                        out=oi_r[:, a:b, :],
                        in_=oi_buf[:, a + 1 : b + 1, :],
                    )

    nc.compile()
    _nc_cache[key] = nc
    return nc


def _shard(X: np.ndarray) -> list[np.ndarray]:
    """[B, T, N] -> per-core [(b j) = 128, T, F] contiguous."""
    Xt = np.ascontiguousarray(
        X.reshape(B, T, J, F).transpose(0, 2, 1, 3)
    )  # [B, J, T, F]
    return [
        Xt[c * BS : (c + 1) * BS].reshape(P, T, F) for c in range(CORES)
    ]


def _unshard(parts: list[np.ndarray]) -> np.ndarray:
    """per-core [(b j), T, F] -> [B, T, N]."""
    full = np.stack(parts).reshape(B, J, T, F)
    return np.ascontiguousarray(full.transpose(0, 2, 1, 3)).reshape(B, T, N)


def _run(X: np.ndarray, w: np.ndarray, **spmd_kwargs):
    X = np.asarray(X, dtype=np.float32)
    inv_tau = float(1.0 / (1.0 + np.exp(-np.float64(np.asarray(w).item()))))
    nc = _build(inv_tau)
    in_maps = [{"x": xs} for xs in _shard(X)]
    res = run_bass_kernel_spmd(nc, in_maps, core_ids=list(range(CORES)), **spmd_kwargs)
    spikes = _unshard(
        [np.asarray(res.results[c]["spk"]).astype(np.float32) for c in range(CORES)]
    )
    i_pot = _unshard(
        [np.asarray(res.results[c]["oi"]).astype(np.float32) for c in range(CORES)]
    )
    return (spikes, i_pot), res


def kernel(X: np.ndarray, w: np.ndarray):
    out, _ = _run(X, w)
    return out


# revision 15
# speedup vs baseline: 1.0115x; 1.0115x over previous
"""AxonLIFNode forward on 8 Trainium2 NeuronCores.

Reference recurrence (per element, sequential over T):
    mem   = mem + (x_t + V_RESET - mem) / TAU        # V_RESET=0, TAU=2
    spike = (mem - V_TH > 0)                         # V_TH=1, {0.0, 1.0}
    mem   = (1 - spike) * mem + V_RESET * spike      # reset to 0 on spike
    out_i = out_i * sigmoid(w) + spike               # axon current (w=0 -> 0.5)
    outputs: (spike, out_i), both [B, T, N] f32

Strategy: data-parallel over the batch axis (B=64 -> 8 per core). Per core the
32768 independent series are laid out as 128 partitions x 256 free elements.
Both recurrences live in linear SBUF buffers with one slot per timestep
(slot 0 = zero initial state) so a single DVE instruction advances a GROUP of
timesteps: for the group's later timesteps the input stream reads the values
the same instruction wrote exactly F=256 elements earlier in the stream --
far beyond the DVE's 8-slice pipeline depth, so the within-instruction RAW is
safe (this is the same pattern the per-group OI op used; here it is applied
to BOTH chains):

    m1_buf[:, t+1, :] = prev + (x_t - prev) * 0.5,  prev = m1_t * (m1_t <= 1)
    oi_buf[:, t+1, :] = oi_t * inv_tau + (m1_{t+1} > 1)

which is bit-exact vs. the reference ordering for m1/spikes (each ALU stage is
one IEEE f32 rounding; *0.5 == /2 exactly). Group sizes taper [1,1,2,4,...]
at the start (compute starts after the first 128 KiB of input) and
[...,4,2,2] at the end (short final dependency chain before the last store).

HBM traffic per core is the wall: X in (8 MiB f32, must stay f32 -- any input
rounding flips spikes), spikes out as fp8-e4m3 ({0,1} lossless, 2 MiB), and
the axon current out as fp16 (4 MiB; the recurrence feedback also runs
through the fp16 buffer, worst-case relative error ~1e-3 << tolerance).
Spikes are produced off the critical path on the Scalar(ACT) engine with a
saturated sigmoid (exact {0,1}, see _build). X streams in on the SP HWDGE
ring; spikes + early oi groups stream out on the ACT ring while late oi
groups move to the SP ring once inputs finish.
"""

import numpy as np

import concourse.bacc as bacc
import concourse.mybir as mybir
import concourse.dve_ops as dve_ops
from concourse.dve_ops import DveOp
from concourse.dve_spec import Spec, Src0, Src1, C0, C1, lower
from concourse.dve_uop import DveOpSpec
from concourse.tile import TileContext
from concourse.bass_utils import run_bass_kernel_spmd

# Problem shape (hardcoded per harness contract).
B, T, N = 64, 64, 4096
CORES = 8
BS = B // CORES          # batches per core
P = 128                  # SBUF partitions
J = 16                   # n-chunks per batch: BS * J == P
F = N // J               # free elements per partition per timestep (256)
SPK_SCALE = 1.0e9        # sigmoid saturation trick scale (see _build)

# Input DMA batches: EXACTLY 8 so each takes one of the 8 global HWDGE
# completion-semaphore lanes at tick 1 -- a 9th+ HWDGE DMA gets a dispatch
# wait for its lane's previous user, and those waits resolve sluggishly
# (measured ~2-4us beyond data completion), serializing the input stream.
BATCH_STARTS = [0, 2, 4, 8, 16, 24, 32, 40, 48, 56, 64]
# M1 group sizes (one DVE instruction each; must nest within input batches).
# Small ramp so compute starts early; small tail so the last chain is short.
GROUPS = [2, 2, 4, 8, 8, 8, 8, 8, 8, 4, 2, 2]
assert sum(GROUPS) == T
# OI / spike / store spans [a, b): emitted once m_buf slots a+1..b are ready.
SPANS = [(0, 4), (4, 8), (8, 24), (24, 40), (40, 48), (48, 56), (56, 62),
         (62, 64)]


def _register_op(name: str, spec: Spec) -> DveOp:
    """Register a custom DVE op in the global registry with a computed sha."""
    for op in dve_ops.OPS:
        if op.name == name:
            return op
    row = dve_ops._CUSTOM_DVE_ROW_BASE + len(dve_ops.OPS)
    assert row < 0x20, "custom-DVE opcode rows exhausted"
    shas = {}
    for ver in ("v3", "v4"):
        uops = lower(spec, ver=ver)
        shas[ver] = DveOpSpec(name=name, opcode=row, uops=uops, rd1_en=True).sha(ver)
    op = DveOp(name, spec, subdim=False, uops_sha=shas)
    dve_ops._SUB_OPCODE_FOR_NAME[name] = row
    dve_ops.OPS.append(op)
    dve_ops.CUSTOM_DVE_SPECS[name] = spec
    return op


def _lif_ops() -> tuple[DveOp, DveOp]:
    """LIF_M1: m1_t from (x_t, m1_{t-1}); LIF_OI: oi_t from (oi_{t-1}, m1_t).

    LIF_M1: out = prev + (Src0 - prev) * C0, prev = Src1 * (Src1 <= C1)
    LIF_OI: out = Src0 * C0 + (Src1 > C1)
    Each ALU stage is one IEEE f32 rounding; bit-exact vs the reference.
    """
    keep = Src1 <= C1
    prev = Src1 * keep
    m1 = _register_op(
        "LIF_M1_ANT",
        Spec(
            body=prev + (Src0 - prev) * C0,
            reference=lambda in0, in1, s0, s1, imm2: (
                (p := (in1 * (in1 <= s1)).astype(np.float32))
                + (in0 - p) * np.float32(s0)
            ).astype(np.float32),
        ),
    )
    oi = _register_op(
        "LIF_OI_ANT",
        Spec(
            body=Src0 * C0 + (Src1 > C1),
            reference=lambda in0, in1, s0, s1, imm2: (
                in0 * np.float32(s0) + (in1 > s1)
            ).astype(np.float32),
        ),
    )
    return m1, oi


_nc_cache: dict = {}


def _build(inv_tau: float):
    """Trace + compile the per-core Bass program (SPMD: same NEFF, 8 cores)."""
    key = float(inv_tau)
    if key in _nc_cache:
        return _nc_cache[key]

    lif_m1, lif_oi = _lif_ops()
    f32 = mybir.dt.float32
    f16 = mybir.dt.float16
    fp8 = mybir.dt.float8e4

    nc = bacc.Bacc(
        "TRN2",
        target_bir_lowering=False,
        debug=False,
        enable_asserts=False,
        num_devices=CORES,
    )
    # Host pre-transposes each core's shard to [(b j) = 128, T, F] contiguous,
    # so every DMA is a 3-dim AP with a contiguous run per partition.
    x_r = nc.dram_tensor("x", [P, T, F], f32, kind="ExternalInput").ap()
    # Spikes are exactly {0.0, 1.0}: store as fp8-e4m3 (lossless) to cut the
    # HBM write traffic 4x; the host upcasts to f32.
    spk_r = nc.dram_tensor("spk", [P, T, F], fp8, kind="ExternalOutput").ap()
    # Axon current as fp16: rel err ~1e-3 (feedback-stable: error recurrence
    # delta_t = 0.5*delta_{t-1} + rounding), host upcasts to f32.
    oi_r = nc.dram_tensor("oi", [P, T, F], f16, kind="ExternalOutput").ap()

    starts = np.cumsum([0] + GROUPS).tolist()

    with TileContext(nc) as tc:
        with (
            tc.tile_pool(name="const", bufs=1) as cpool,
            tc.tile_pool(name="xin", bufs=1) as xpool,
            tc.tile_pool(name="sout", bufs=6) as spool,
            tc.tile_pool(name="state", bufs=1) as mpool,
        ):
            # Spike via one ACT op: sigmoid(S*m1 - (S + 64)) with S = 1e9.
            # fl(S*m1) quantizes to a 64-ulp grid around S, so the argument is
            # always <= -64 (no spike, incl. m1 == V_TH exactly -> -64) or
            # >= +64 (spike); sigmoid saturates to 0.0 / 1.0 there.
            spk_bias = cpool.tile([P, 1], f32)
            nc.gpsimd.memset(spk_bias[:], -(SPK_SCALE + 64.0))

            # Linear state buffers: slot 0 is the zero initial state, value
            # for timestep t lives at slot t+1.
            m_buf = mpool.tile([P, T + 1, F], f32)
            oi_buf = mpool.tile([P, T + 1, F], f16)
            nc.gpsimd.memset(m_buf[:, 0, :], 0.0)
            nc.gpsimd.memset(oi_buf[:, 0, :], 0.0)

            # Per-batch X tiles. All 8 input DMAs are emitted at the highest
            # scheduler priority so they claim the 8 HWDGE lanes at tick 1
            # (zero dispatch waits) and stream FIFO on the SP ring.
            x_tiles = [
                xpool.tile(
                    [P, BATCH_STARTS[i + 1] - BATCH_STARTS[i], F],
                    f32,
                    name=f"x_{BATCH_STARTS[i]}",
                    bufs=1,
                )
                for i in range(len(BATCH_STARTS) - 1)
            ]
            # SDMA engines round-robin across ALL queued DMAs at packet
            # granularity, so enqueueing every batch at once makes the batch
            # the DVE is waiting on share the wire with batches needed much
            # later. Pace the dispatch of later batches with manual
            # timestamps so early batches drain at full rate.
            x_dispatch_ms = {4: 0.016, 5: 0.0185, 6: 0.020, 7: 0.0215,
                             8: 0.023, 9: 0.0245}
            with tc.high_priority():
                for bi in range(len(x_tiles)):
                    with tc.tile_wait_until(
                        x_dispatch_ms.get(bi, 0), enable=bi in x_dispatch_ms
                    ):
                        nc.sync.dma_start(
                            out=x_tiles[bi][:],
                            in_=x_r[:, BATCH_STARTS[bi] : BATCH_STARTS[bi + 1], :],
                        )

            def x_slice(t0, g):
                for bi in range(len(x_tiles)):
                    if BATCH_STARTS[bi] <= t0 and t0 + g <= BATCH_STARTS[bi + 1]:
                        lo = t0 - BATCH_STARTS[bi]
                        return x_tiles[bi][:, lo : lo + g, :]
                raise AssertionError((t0, g))

            span_idx = 0
            for gi, g in enumerate(GROUPS):
                t0 = starts[gi]
                # DVE: advance the membrane chain g timesteps in one op.
                nc.vector._custom_dve(
                    lif_m1,
                    out=m_buf[:, t0 + 1 : t0 + 1 + g, :],
                    in0=x_slice(t0, g),
                    in1=m_buf[:, t0 : t0 + g, :],
                    s0=0.5,      # 1/TAU
                    s1=1.0,      # V_TH
                )
                while span_idx < len(SPANS) and SPANS[span_idx][1] == t0 + g:
                    a, b = SPANS[span_idx]
                    span_idx += 1
                    # ACT: spike = sigmoid(S*m1 - (S+64)) in {0, 1}, one
                    # op/span, written as fp8 (exact for {0,1}).
                    s_t = spool.tile([P, b - a, F], fp8)
                    nc.scalar.activation(
                        out=s_t[:],
                        in_=m_buf[:, a + 1 : b + 1, :],
                        func=mybir.ActivationFunctionType.Sigmoid,
                        bias=spk_bias[:],
                        scale=SPK_SCALE,
                    )
                    nc.scalar.dma_start(out=spk_r[:, a:b, :], in_=s_t[:])
                    # DVE: advance the axon-current chain b-a timesteps in
                    # one op (fp16 in/out; ALU math is fp32 internally).
                    nc.vector._custom_dve(
                        lif_oi,
                        out=oi_buf[:, a + 1 : b + 1, :],
                        in0=oi_buf[:, a : b, :],
                        in1=m_buf[:, a + 1 : b + 1, :],
                        s0=inv_tau,
                        s1=1.0,
                    )
                    oi_eng = nc.scalar if b <= 40 else nc.sync
                    oi_eng.dma_start(
                        out=oi_r[:, a:b, :],
                        in_=oi_buf[:, a + 1 : b + 1, :],
                    )

    nc.compile()
    _nc_cache[key] = nc
    return nc


def _shard(X: np.ndarray) -> list[np.ndarray]:
    """[B, T, N] -> per-core [(b j) = 128, T, F] contiguous."""
    Xt = np.ascontiguousarray(
        X.reshape(B, T, J, F).transpose(0, 2, 1, 3)
    )  # [B, J, T, F]
    return [
        Xt[c * BS : (c + 1) * BS].reshape(P, T, F) for c in range(CORES)
    ]


def _unshard(parts: list[np.ndarray]) -> np.ndarray:
    """per-core [(b j), T, F] -> [B, T, N]."""
    full = np.stack(parts).reshape(B, J, T, F)
    return np.ascontiguousarray(full.transpose(0, 2, 1, 3)).reshape(B, T, N)


def _run(X: np.ndarray, w: np.ndarray, **spmd_kwargs):
    X = np.asarray(X, dtype=np.float32)
    inv_tau = float(1.0 / (1.0 + np.exp(-np.float64(np.asarray(w).item()))))
    nc = _build(inv_tau)
    in_maps = [{"x": xs} for xs in _shard(X)]
    res = run_bass_kernel_spmd(nc, in_maps, core_ids=list(range(CORES)), **spmd_kwargs)
    spikes = _unshard(
        [np.asarray(res.results[c]["spk"]).astype(np.float32) for c in range(CORES)]
    )
    i_pot = _unshard(
        [np.asarray(res.results[c]["oi"]).astype(np.float32) for c in range(CORES)]
    )
    return (spikes, i_pot), res


def kernel(X: np.ndarray, w: np.ndarray):
    out, _ = _run(X, w)
    return out


# revision 16
# speedup vs baseline: 1.0199x; 1.0083x over previous
"""AxonLIFNode forward on 8 Trainium2 NeuronCores.

Reference recurrence (per element, sequential over T):
    mem   = mem + (x_t + V_RESET - mem) / TAU        # V_RESET=0, TAU=2
    spike = (mem - V_TH > 0)                         # V_TH=1, {0.0, 1.0}
    mem   = (1 - spike) * mem + V_RESET * spike      # reset to 0 on spike
    out_i = out_i * sigmoid(w) + spike               # axon current (w=0 -> 0.5)
    outputs: (spike, out_i), both [B, T, N] f32

Strategy: data-parallel over the batch axis (B=64 -> 8 per core). Per core the
32768 independent series are laid out as 128 partitions x 256 free elements.
Both recurrences live in linear SBUF buffers with one slot per timestep
(slot 0 = zero initial state) so a single DVE instruction advances a GROUP of
timesteps: for the group's later timesteps the input stream reads the values
the same instruction wrote exactly F=256 elements earlier in the stream --
far beyond the DVE's 8-slice pipeline depth, so the within-instruction RAW is
safe (this is the same pattern the per-group OI op used; here it is applied
to BOTH chains):

    m1_buf[:, t+1, :] = prev + (x_t - prev) * 0.5,  prev = m1_t * (m1_t <= 1)
    oi_buf[:, t+1, :] = oi_t * inv_tau + (m1_{t+1} > 1)

which is bit-exact vs. the reference ordering for m1/spikes (each ALU stage is
one IEEE f32 rounding; *0.5 == /2 exactly). Group sizes taper [1,1,2,4,...]
at the start (compute starts after the first 128 KiB of input) and
[...,4,2,2] at the end (short final dependency chain before the last store).

HBM traffic per core is the wall: X in (8 MiB f32, must stay f32 -- any input
rounding flips spikes), spikes out as fp8-e4m3 ({0,1} lossless, 2 MiB), and
the axon current out as fp16 (4 MiB; the recurrence feedback also runs
through the fp16 buffer, worst-case relative error ~1e-3 << tolerance).
Spikes are produced off the critical path on the Scalar(ACT) engine with a
saturated sigmoid (exact {0,1}, see _build). X streams in on the SP HWDGE
ring; spikes + early oi groups stream out on the ACT ring while late oi
groups move to the SP ring once inputs finish.
"""

import numpy as np

import concourse.bacc as bacc
import concourse.mybir as mybir
import concourse.dve_ops as dve_ops
from concourse.dve_ops import DveOp
from concourse.dve_spec import Spec, Src0, Src1, C0, C1, lower
from concourse.dve_uop import DveOpSpec
from concourse.tile import TileContext
from concourse.bass_utils import run_bass_kernel_spmd

# Problem shape (hardcoded per harness contract).
B, T, N = 64, 64, 4096
CORES = 8
BS = B // CORES          # batches per core
P = 128                  # SBUF partitions
J = 16                   # n-chunks per batch: BS * J == P
F = N // J               # free elements per partition per timestep (256)
SPK_SCALE = 1.0e9        # sigmoid saturation trick scale (see _build)

# Input DMA batches: EXACTLY 8 so each takes one of the 8 global HWDGE
# completion-semaphore lanes at tick 1 -- a 9th+ HWDGE DMA gets a dispatch
# wait for its lane's previous user, and those waits resolve sluggishly
# (measured ~2-4us beyond data completion), serializing the input stream.
BATCH_STARTS = [0, 4, 8, 16, 24, 32, 40, 48, 56, 64]
# M1 group sizes (one DVE instruction each; must nest within input batches).
# Small ramp so compute starts early; small tail so the last chain is short.
GROUPS = [4, 4, 8, 8, 8, 8, 8, 8, 4, 2, 2]
assert sum(GROUPS) == T
# OI / spike / store spans [a, b): emitted once m_buf slots a+1..b are ready.
SPANS = [(0, 4), (4, 8), (8, 24), (24, 40), (40, 48), (48, 56), (56, 62),
         (62, 64)]


def _register_op(name: str, spec: Spec) -> DveOp:
    """Register a custom DVE op in the global registry with a computed sha."""
    for op in dve_ops.OPS:
        if op.name == name:
            return op
    row = dve_ops._CUSTOM_DVE_ROW_BASE + len(dve_ops.OPS)
    assert row < 0x20, "custom-DVE opcode rows exhausted"
    shas = {}
    for ver in ("v3", "v4"):
        uops = lower(spec, ver=ver)
        shas[ver] = DveOpSpec(name=name, opcode=row, uops=uops, rd1_en=True).sha(ver)
    op = DveOp(name, spec, subdim=False, uops_sha=shas)
    dve_ops._SUB_OPCODE_FOR_NAME[name] = row
    dve_ops.OPS.append(op)
    dve_ops.CUSTOM_DVE_SPECS[name] = spec
    return op


def _lif_ops() -> tuple[DveOp, DveOp]:
    """LIF_M1: m1_t from (x_t, m1_{t-1}); LIF_OI: oi_t from (oi_{t-1}, m1_t).

    LIF_M1: out = prev + (Src0 - prev) * C0, prev = Src1 * (Src1 <= C1)
    LIF_OI: out = Src0 * C0 + (Src1 > C1)
    Each ALU stage is one IEEE f32 rounding; bit-exact vs the reference.
    """
    keep = Src1 <= C1
    prev = Src1 * keep
    m1 = _register_op(
        "LIF_M1_ANT",
        Spec(
            body=prev + (Src0 - prev) * C0,
            reference=lambda in0, in1, s0, s1, imm2: (
                (p := (in1 * (in1 <= s1)).astype(np.float32))
                + (in0 - p) * np.float32(s0)
            ).astype(np.float32),
        ),
    )
    oi = _register_op(
        "LIF_OI_ANT",
        Spec(
            body=Src0 * C0 + (Src1 > C1),
            reference=lambda in0, in1, s0, s1, imm2: (
                in0 * np.float32(s0) + (in1 > s1)
            ).astype(np.float32),
        ),
    )
    return m1, oi


_nc_cache: dict = {}


def _build(inv_tau: float):
    """Trace + compile the per-core Bass program (SPMD: same NEFF, 8 cores)."""
    key = float(inv_tau)
    if key in _nc_cache:
        return _nc_cache[key]

    lif_m1, lif_oi = _lif_ops()
    f32 = mybir.dt.float32
    f16 = mybir.dt.float16
    fp8 = mybir.dt.float8e4

    nc = bacc.Bacc(
        "TRN2",
        target_bir_lowering=False,
        debug=False,
        enable_asserts=False,
        num_devices=CORES,
    )
    # Host pre-transposes each core's shard to [(b j) = 128, T, F] contiguous,
    # so every DMA is a 3-dim AP with a contiguous run per partition.
    x_r = nc.dram_tensor("x", [P, T, F], f32, kind="ExternalInput").ap()
    # Spikes are exactly {0.0, 1.0}: store as fp8-e4m3 (lossless) to cut the
    # HBM write traffic 4x; the host upcasts to f32.
    spk_r = nc.dram_tensor("spk", [P, T, F], fp8, kind="ExternalOutput").ap()
    # Axon current as fp16: rel err ~1e-3 (feedback-stable: error recurrence
    # delta_t = 0.5*delta_{t-1} + rounding), host upcasts to f32.
    oi_r = nc.dram_tensor("oi", [P, T, F], f16, kind="ExternalOutput").ap()

    starts = np.cumsum([0] + GROUPS).tolist()

    with TileContext(nc) as tc:
        with (
            tc.tile_pool(name="const", bufs=1) as cpool,
            tc.tile_pool(name="xin", bufs=1) as xpool,
            tc.tile_pool(name="sout", bufs=6) as spool,
            tc.tile_pool(name="state", bufs=1) as mpool,
        ):
            # Spike via one ACT op: sigmoid(S*m1 - (S + 64)) with S = 1e9.
            # fl(S*m1) quantizes to a 64-ulp grid around S, so the argument is
            # always <= -64 (no spike, incl. m1 == V_TH exactly -> -64) or
            # >= +64 (spike); sigmoid saturates to 0.0 / 1.0 there.
            spk_bias = cpool.tile([P, 1], f32)
            nc.gpsimd.memset(spk_bias[:], -(SPK_SCALE + 64.0))

            # Linear state buffers: slot 0 is the zero initial state, value
            # for timestep t lives at slot t+1.
            m_buf = mpool.tile([P, T + 1, F], f32)
            oi_buf = mpool.tile([P, T + 1, F], f16)
            nc.gpsimd.memset(m_buf[:, 0, :], 0.0)
            nc.gpsimd.memset(oi_buf[:, 0, :], 0.0)

            # Per-batch X tiles. All 8 input DMAs are emitted at the highest
            # scheduler priority so they claim the 8 HWDGE lanes at tick 1
            # (zero dispatch waits) and stream FIFO on the SP ring.
            x_tiles = [
                xpool.tile(
                    [P, BATCH_STARTS[i + 1] - BATCH_STARTS[i], F],
                    f32,
                    name=f"x_{BATCH_STARTS[i]}",
                    bufs=1,
                )
                for i in range(len(BATCH_STARTS) - 1)
            ]
            # SDMA engines round-robin across ALL queued DMAs at packet
            # granularity, so enqueueing every batch at once makes the batch
            # the DVE is waiting on share the wire with batches needed much
            # later. Pace the dispatch of later batches with manual
            # timestamps so early batches drain at full rate.
            x_dispatch_ms = {3: 0.022, 4: 0.024, 5: 0.0255, 6: 0.027,
                             7: 0.0285, 8: 0.030}
            with tc.high_priority():
                for bi in range(len(x_tiles)):
                    with tc.tile_wait_until(
                        x_dispatch_ms.get(bi, 0), enable=bi in x_dispatch_ms
                    ):
                        nc.sync.dma_start(
                            out=x_tiles[bi][:],
                            in_=x_r[:, BATCH_STARTS[bi] : BATCH_STARTS[bi + 1], :],
                        )

            def x_slice(t0, g):
                for bi in range(len(x_tiles)):
                    if BATCH_STARTS[bi] <= t0 and t0 + g <= BATCH_STARTS[bi + 1]:
                        lo = t0 - BATCH_STARTS[bi]
                        return x_tiles[bi][:, lo : lo + g, :]
                raise AssertionError((t0, g))

            span_idx = 0
            for gi, g in enumerate(GROUPS):
                t0 = starts[gi]
                # DVE: advance the membrane chain g timesteps in one op.
                nc.vector._custom_dve(
                    lif_m1,
                    out=m_buf[:, t0 + 1 : t0 + 1 + g, :],
                    in0=x_slice(t0, g),
                    in1=m_buf[:, t0 : t0 + g, :],
                    s0=0.5,      # 1/TAU
                    s1=1.0,      # V_TH
                )
                while span_idx < len(SPANS) and SPANS[span_idx][1] == t0 + g:
                    a, b = SPANS[span_idx]
                    span_idx += 1
                    # ACT: spike = sigmoid(S*m1 - (S+64)) in {0, 1}, one
                    # op/span, written as fp8 (exact for {0,1}).
                    s_t = spool.tile([P, b - a, F], fp8)
                    nc.scalar.activation(
                        out=s_t[:],
                        in_=m_buf[:, a + 1 : b + 1, :],
                        func=mybir.ActivationFunctionType.Sigmoid,
                        bias=spk_bias[:],
                        scale=SPK_SCALE,
                    )
                    nc.scalar.dma_start(out=spk_r[:, a:b, :], in_=s_t[:])
                    # DVE: advance the axon-current chain b-a timesteps in
                    # one op (fp16 in/out; ALU math is fp32 internally).
                    nc.vector._custom_dve(
                        lif_oi,
                        out=oi_buf[:, a + 1 : b + 1, :],
                        in0=oi_buf[:, a : b, :],
                        in1=m_buf[:, a + 1 : b + 1, :],
                        s0=inv_tau,
                        s1=1.0,
                    )
                    oi_eng = nc.scalar if b <= 40 else nc.sync
                    oi_eng.dma_start(
                        out=oi_r[:, a:b, :],
                        in_=oi_buf[:, a + 1 : b + 1, :],
                    )

    nc.compile()
    _nc_cache[key] = nc
    return nc


def _shard(X: np.ndarray) -> list[np.ndarray]:
    """[B, T, N] -> per-core [(b j) = 128, T, F] contiguous."""
    Xt = np.ascontiguousarray(
        X.reshape(B, T, J, F).transpose(0, 2, 1, 3)
    )  # [B, J, T, F]
    return [
        Xt[c * BS : (c + 1) * BS].reshape(P, T, F) for c in range(CORES)
    ]


def _unshard(parts: list[np.ndarray]) -> np.ndarray:
    """per-core [(b j), T, F] -> [B, T, N]."""
    full = np.stack(parts).reshape(B, J, T, F)
    return np.ascontiguousarray(full.transpose(0, 2, 1, 3)).reshape(B, T, N)


def _run(X: np.ndarray, w: np.ndarray, **spmd_kwargs):
    X = np.asarray(X, dtype=np.float32)
    inv_tau = float(1.0 / (1.0 + np.exp(-np.float64(np.asarray(w).item()))))
    nc = _build(inv_tau)
    in_maps = [{"x": xs} for xs in _shard(X)]
    res = run_bass_kernel_spmd(nc, in_maps, core_ids=list(range(CORES)), **spmd_kwargs)
    spikes = _unshard(
        [np.asarray(res.results[c]["spk"]).astype(np.float32) for c in range(CORES)]
    )
    i_pot = _unshard(
        [np.asarray(res.results[c]["oi"]).astype(np.float32) for c in range(CORES)]
    )
    return (spikes, i_pot), res


def kernel(X: np.ndarray, w: np.ndarray):
    out, _ = _run(X, w)
    return out
